# revision 28
# baseline (speedup 1.0000x reference)
"""Distributed Trainium2 kernel for AdaptiveLinearWithChannel (MoE-routed
batched matmul):  out[t] = x[t] @ weight[indices[t]] + bias

Expert-parallel per the sharding hint: the tile dimension is sharded
64-tiles-per-core across 8 NeuronCores; the indices gather is resolved
host-side (each core receives its 64 x tiles plus its 64 pre-gathered
weight tiles), so routing is device-local and no collectives run.

Quantization (l2 rel err 1.679e-2 vs the f32 reference; gate 2e-2):
  x   -> float8_e3m4, x2 pow2 pre-scale          (8 MiB/core, Sync q1)
  w   -> int8 in HBM (w is kaiming-UNIFORM, so int8 rel err is only
         3.9e-3 vs e3m4's 1.3e-2), CAST to bf16 during the GpSimd
         SWDGE DMA — exact, |w_int|<=127          (4 MiB/core, q0)
  out -> int8, per-tile scale so[t]=127/(4*sigma_est); the DVE/ACT
         f32->int8 converts are RNE+saturating (HW-probed)
                                                  (8 MiB/core, ACT q10)
  PE runs its bf16(lhsT) x e3m4(rhs) path with exact f32 PSUM
  accumulation (products have <=13-bit mantissas).

Why this shape (from NTFF traces of each revision):
  - PE path: 256 matmuls x 512 cols / 2.4 GHz = 54.6 us. fp8 Double
    modes need e4m3/e5m2 operands (3-bit mantissa) — precision rules
    them out, so this is the compute floor.
  - SDMA fabric path: ~425 GB/s of SBUF-side bytes, shared by all 3
    DMA queues; the w cast-DMA costs WRITE-side bytes, so the total is
    25.2 MiB ~= 59.3 us — the binding path. On-chip upcasts (DVE/ACT)
    to cut it were measured SLOWER (ACT busy grew past the PE path);
    per-core weight dedup is impossible under SPMD (one program).
  - Wall ~= 8 us preamble/issue + ~60-63 us fabric-paced stream +
    ~4 us tail + ~3 us of the fixed ~7 us semaphore-reset exit ladder.
  - PE HAM warm-up: ~12 N=512 scratch matmuls fill the DMA ramp so the
    real stream starts at 2.4 GHz (HAM un-throttles after ~3.4 us of
    sustained activity; without this the first ~5 us ran at 1.2 GHz).
  - x/w SBUF buffers are fully resident (one per super-tile, SUP=4
    tiles each): SP/GpSimd issue all in-DMAs upfront, PE waits only
    per-super DMA sems, DVE (even tiles) and ACT (odd tiles) copy
    PSUM->int8 through 4 rotating PSUM bank pairs, and out-DMAs ride
    ACT q10 (supers 0-11), SP q1 (12-14, after its x stream), with the
    last super split per-tile over ACT/SP so the drain chases copies.

A full-coverage column-sum integrity check retries the rare transient
device corruption. Block exit skips gpsimd's SWDGE dge_drain (its
w-DMAs are all consumed by compute; SP/ACT drain their own queues,
which carry every out-DMA).
"""

import numpy as np
import ml_dtypes

import concourse.bass as bass
import concourse.mybir as mybir
from concourse.bass_utils import run_bass_kernel_spmd

BF16 = ml_dtypes.bfloat16
E3M4 = ml_dtypes.float8_e3m4

N_CORES = 8
NUM_TILES = 512
N_POINTS = 512          # free dim N of each matmul
D_IN = 256              # contraction, 2 chunks of 128
D_OUT = 256             # output partitions, 2 chunks of 128
CHANNELS = 1024
TPC = NUM_TILES // N_CORES   # 64 tiles per core
SUP = 4                      # tiles per super-tile (4 KiB DMA rows)
NSUP = TPC // SUP            # 16 super-tiles per core
NBUF = 8                     # o_sb buffer sets (out pipeline depth)
# x/w are FULLY RESIDENT (one SBUF buffer per super, ~16 MiB): SP/GpSimd
# issue every in-DMA upfront with no inter-engine gating, so the in
# queues run at their solo rate early and build enough lead to absorb
# the 3-queue-contention deficit that caused mid-stream PE gaps.
GROUPS_PER_SUP = SUP * 2     # psum groups (sem_pe incs) per super-tile

OUT_MARGIN = 4.0             # int8 out scale: so[t] = 127/(margin*sigma)
PE_WARM = 12                 # N=512 scratch matmuls during the DMA ramp

_cache = {}

# odd supers are plain-int8 + on-chip upcast; engine alternates DVE/ACT.
# DISABLED (empty): a 94.6us A/B showed the upcasts push ACT to ~63us
# busy (copies 38 + casts 10 + DMA issues 9) > the 54.6us PE path, so
# the whole stream stretched; the SWDGE cast-DMA fabric cost is cheaper.
# ALSO measured slower (83.3us vs 80.2): 8 front-loaded DVE upcasts make
# DVE the first-half pacer (2 copies + 1 cast = 3.79us/super > PE's 3.5).
CONV_SUPERS = []
CONV_ENGINE = {s: 'v' for s in CONV_SUPERS}
CONV_IDX = {s: i for i, s in enumerate(CONV_SUPERS)}

# out-DMA routing: super -> issuing engine; last super split per-tile
OUT_OWNER = {s: 'a' for s in range(12)}
OUT_OWNER.update({12: 's', 13: 's', 14: 's'})
TAIL_OWNER = ['a', 's', 's', 'a']            # t2 -> engine for s=15 (HWDGE only)


def _build_nc():
    bf = mybir.dt.bfloat16
    f32 = mybir.dt.float32
    e3 = mybir.dt.float8e3
    i8 = mybir.dt.int8
    nc = bass.Bass()

    # x_dev[s, p, c, t2, f]    = x[4s+t2, f, 128c+p] * SX     (e3m4)
    # w_dev[s, p, c, t2, o]    = w_int8[4s+t2, 128c+p, o]     (int8)
    # out_dev[s, po, t2, j, f] = out[4s+t2, f, 128j+po] * so[t] (int8)
    # aux_dev[p, t] = so[t]/(SX*SW)  (psum multiplier)
    x_ext = nc.declare_dram_parameter("x", [NSUP // 2, 128, 2, 2 * SUP, N_POINTS], e3, isOutput=False)
    w_ext = nc.declare_dram_parameter("w", [NSUP // 2, 128, 2, 2 * SUP, D_OUT], i8, isOutput=False)
    aux_ext = nc.declare_dram_parameter("aux", [128, TPC], f32, isOutput=False)
    out_ext = nc.declare_dram_parameter("out", [NSUP, 128, SUP, 2, N_POINTS], i8, isOutput=True)

    import contextlib
    ctx = contextlib.ExitStack()
    x_sb = [ctx.enter_context(nc.sbuf_tensor(f"x_sb{i}", [128, 2, 2 * SUP, N_POINTS], e3)) for i in range(NSUP // 2)]
    w_sb = [ctx.enter_context(nc.sbuf_tensor(f"w_sb{i}", [128, 2, 2 * SUP, D_OUT], bf)) for i in range(NSUP // 2)]
    o_sb = [ctx.enter_context(nc.sbuf_tensor(f"o_sb{i}", [128, SUP, 2, N_POINTS], i8)) for i in range(NBUF)]
    w8_sb = {c: ctx.enter_context(nc.sbuf_tensor(f"w8_sb{c}", [128, 2, SUP, D_OUT], i8)) for c in CONV_SUPERS}
    aux_sb = ctx.enter_context(nc.sbuf_tensor("aux_sb", [128, TPC], f32))
    scr_sb = ctx.enter_context(nc.sbuf_tensor("scr_sb", [128, 1], f32))
    # scratch operands for PE HAM warm-up matmuls (uninitialized is fine)
    dmw_sb = ctx.enter_context(nc.sbuf_tensor("dmw_sb", [128, 128], bf))
    dmx_sb = ctx.enter_context(nc.sbuf_tensor("dmx_sb", [128, N_POINTS], e3))
    # 4 bank-pairs: tile t uses pair t%4, one [128,1024] copy per tile
    psum = [ctx.enter_context(nc.psum_tensor(f"ps{i}", [128, 2 * N_POINTS], f32)) for i in range(4)]

    # sem_xw totals per use: plain supers get x(16)+w cast-dma(16)=32;
    # conv supers get x(16) only — the upcast completion is tracked on its
    # own engine sem (sem_cvd/sem_cva) so DMA sems never mix DGE and
    # engine increments. Slot b serves supers b, b+8 (same parity =>
    # same kind).
    def xw_total(s):
        per_use = 16 if s in CONV_IDX else 32
        return per_use * (s // NBUF + 1)

    # cast-completion threshold PE must wait for before conv super s
    def cast_wait(s):
        eng = CONV_ENGINE[s]
        cnt = sum(1 for c in CONV_SUPERS if c <= s and CONV_ENGINE[c] == eng)
        return eng, cnt

    with ctx:
        with (
            contextlib.ExitStack() as sems,
            nc.Block(no_gpsimd_drain=True) as block,
        ):
            sem_xw = [sems.enter_context(nc.semaphore(f"sem_xw{b}")) for b in range(NSUP)]
            sem_w2 = [sems.enter_context(nc.semaphore(f"sem_w2{q}")) for q in range(NSUP // 2)]
            sem_w1 = sems.enter_context(nc.semaphore("sem_w1"))
            sem_w8 = {c: sems.enter_context(nc.semaphore(f"sem_w8{c}")) for c in CONV_SUPERS}
            sem_o = [sems.enter_context(nc.semaphore(f"sem_o{b}")) for b in range(NBUF)]
            sem_aux = sems.enter_context(nc.semaphore("sem_aux"))
            sem_pe = sems.enter_context(nc.semaphore("sem_pe"))
            sem_dve = sems.enter_context(nc.semaphore("sem_dve"))
            sem_acp = sems.enter_context(nc.semaphore("sem_acp"))
            sem_cvd = sems.enter_context(nc.semaphore("sem_cvd"))   # DVE upcasts
            sem_cva = sems.enter_context(nc.semaphore("sem_cva"))   # ACT upcasts

            # per-tile copy completion: DVE even t2, ACT odd t2
            def copy_done(t):
                return (sem_dve, t // 2 + 1) if t % 2 == 0 else (sem_acp, t // 2 + 1)

            def super_copies_done(eng, s):
                eng.wait_ge(sem_dve, (SUP // 2) * (s + 1))
                eng.wait_ge(sem_acp, (SUP // 2) * (s + 1))

            # upcast completion for conv index i (0-based over CONV_SUPERS)
            def conv_done(i):
                s = CONV_SUPERS[i]
                eng = CONV_ENGINE[s]
                cnt = sum(1 for c in CONV_SUPERS[:i + 1] if CONV_ENGINE[c] == eng)
                return (sem_cvd if eng == 'v' else sem_cva, cnt)

            # upcasts are placed ~2 supers ahead of PE's need: conv super c
            # is cast while the engine sits at loop step s = max(0, c - 2)
            def conv_at(s, eng):
                for c in CONV_SUPERS:
                    if CONV_ENGINE[c] == eng and max(0, c - 2) == s:
                        yield c

            @block.sync
            def _(sp):
                sp.dma_start(x_sb[0][:, :, 0:SUP, :], x_ext[0][:, :, 0:SUP, :]
                             ).then_inc(sem_xw[0], 16)
                sp.dma_start(x_sb[0][:, :, SUP:2 * SUP, :], x_ext[0][:, :, SUP:2 * SUP, :]
                             ).then_inc(sem_xw[1], 16)
                for q in range(1, NSUP // 2):
                    sp.dma_start(x_sb[q][:], x_ext[q]).then_inc(sem_xw[2 * q], 16)
                for s in (12, 13, 14):
                    super_copies_done(sp, s)
                    sp.dma_start(out_ext[s], o_sb[s % NBUF][:]).then_inc(sem_o[s % NBUF], 16)
                s = NSUP - 1
                for t2 in range(SUP):
                    if TAIL_OWNER[t2] != 's':
                        continue
                    t = s * SUP + t2
                    csem, ccnt = copy_done(t)
                    sp.wait_ge(csem, ccnt)
                    sp.dma_start(out_ext[s][:, t2], o_sb[s % NBUF][:, t2]
                                 ).then_inc(sem_o[s % NBUF], 16)


            @block.gpsimd
            def _(gp):
                gp.dma_start(aux_sb[:], aux_ext[:]).then_inc(sem_aux, 16)
                # supers 0/1 as singles (small first transfers keep the PE
                # start early); supers 2-15 as pair-DMAs: 4 KiB HBM rows
                # measured 178 GB/s on this queue vs 136 for 2 KiB rows
                gp.dma_start(w_sb[0][:, :, 0:SUP, :], w_ext[0][:, :, 0:SUP, :]
                             ).then_inc(sem_w2[0], 16)
                gp.dma_start(w_sb[0][:, :, SUP:2 * SUP, :], w_ext[0][:, :, SUP:2 * SUP, :]
                             ).then_inc(sem_w1, 16)
                for q in range(1, NSUP // 2):
                    gp.dma_start(w_sb[q][:], w_ext[q]).then_inc(sem_w2[q], 16)


            @block.tensor
            def _(pe):
                # HAM warm-up: PE clock ramps 1.2->2.4 GHz after ~3.4us of
                # sustained activity; fill the DMA ramp (~5.5us) with long
                # scratch matmuls so the real stream starts warm.
                for _k in range(PE_WARM):
                    pe.matmul(psum[3][:, 0:512], dmw_sb[:], dmx_sb[:],
                              start=True, stop=True, skip_group_check=True)
                for s in range(NSUP):
                    pe.wait_ge(sem_xw[s if s < 2 else 2 * (s // 2)], 16)
                    if s == 1:
                        pe.wait_ge(sem_w1, 16)
                    else:
                        pe.wait_ge(sem_w2[s // 2], 16)
                    for t2 in range(SUP):
                        t = s * SUP + t2
                        if t >= 4:
                            csem, ccnt = copy_done(t - 4)
                            pe.wait_ge(csem, ccnt)
                        ps = psum[t % 4]
                        u = (s % 2) * SUP + t2
                        for j in range(2):
                            pe.matmul(ps[:, j * 512:(j + 1) * 512],
                                      w_sb[s // 2][:, 0, u, j * 128:(j + 1) * 128],
                                      x_sb[s // 2][:, 0, u, :], start=True, stop=False)
                            pe.matmul(ps[:, j * 512:(j + 1) * 512],
                                      w_sb[s // 2][:, 1, u, j * 128:(j + 1) * 128],
                                      x_sb[s // 2][:, 1, u, :], start=False, stop=True
                                      ).then_inc(sem_pe, 1)

            @block.vector
            def _(dve):
                dve.wait_ge(sem_aux, 16)
                for s in range(NSUP):
                    b = s % NBUF
                    for c in conv_at(s, 'v'):
                        dve.wait_ge(sem_w8[c], 16)
                        dve.tensor_copy(w_sb[c][:], w8_sb[c][:]).then_inc(sem_cvd, 1)
                    if s >= NBUF:
                        dve.wait_ge(sem_o[b], 16)
                    for t2 in range(0, SUP, 2):
                        t = s * SUP + t2
                        dve.wait_ge(sem_pe, 2 * t + 2)
                        dve.tensor_scalar_mul(o_sb[b][:, t2, :, :], psum[t % 4][:],
                                              aux_sb[:, t:t + 1]).then_inc(sem_dve, 1)

            @block.scalar
            def _(act):
                # plain-int8 staging DMAs for the DVE-upcast supers ride
                # ACT's queue, which otherwise idles until the first out
                for c in CONV_SUPERS:
                    act.dma_start(w8_sb[c][:], w_ext[c]).then_inc(sem_w8[c], 16)
                act.wait_ge(sem_aux, 16)
                # dummy activation: pulls the lazy ACT_TABLE_LOAD into the
                # preamble instead of serializing before the first copy
                act.activation(scr_sb[:], aux_sb[:, 0:1],
                               mybir.ActivationFunctionType.Copy)
                for s in range(NSUP):
                    b = s % NBUF
                    if s >= NBUF:
                        act.wait_ge(sem_o[b], 16)
                    for t2 in range(1, SUP, 2):
                        t = s * SUP + t2
                        act.wait_ge(sem_pe, 2 * t + 2)
                        act.activation(o_sb[b][:, t2, :, :], psum[t % 4][:],
                                       mybir.ActivationFunctionType.Copy,
                                       scale=aux_sb[:, t:t + 1]).then_inc(sem_acp, 1)
                    if OUT_OWNER.get(s) == 'a':
                        act.wait_ge(sem_acp, (SUP // 2) * (s + 1))
                        act.wait_ge(sem_dve, (SUP // 2) * (s + 1))
                        act.dma_start(out_ext[s], o_sb[b][:]).then_inc(sem_o[b], 16)
                s = NSUP - 1
                for t2 in range(SUP):
                    if TAIL_OWNER[t2] != 'a':
                        continue
                    t = s * SUP + t2
                    csem, ccnt = copy_done(t)
                    act.wait_ge(csem, ccnt)
                    act.dma_start(out_ext[s][:, t2], o_sb[s % NBUF][:, t2]
                                  ).then_inc(sem_o[s % NBUF], 16)

    return nc


def _quant_x(x_f32):
    """Pick a pow2 pre-scale keeping |x*SX| comfortably under e3m4 max."""
    amax = float(np.abs(x_f32).max()) + 1e-30
    return 2.0 ** int(np.floor(np.log2(14.0 / amax)))


def _pack_core(x_core_f32, w_gathered_i8, so_core, sx, sw):
    """Host-side repack of one core's shard into the device in_map."""
    x8 = (x_core_f32 * sx).astype(E3M4)                # [64, 512, 256]
    x_dev = np.ascontiguousarray(
        x8.reshape(NSUP // 2, 2 * SUP, N_POINTS, 2, 128).transpose(0, 4, 3, 1, 2))
    w_dev = np.ascontiguousarray(
        w_gathered_i8.reshape(NSUP // 2, 2 * SUP, 2, 128, D_OUT).transpose(0, 3, 2, 1, 4))
    aux = np.ascontiguousarray(
        np.broadcast_to((so_core / (sx * sw))[None, :], (128, TPC))).astype(np.float32)
    return {"x": x_dev, "w": w_dev, "aux": aux}


def _unpack_core(out_dev, so_core):
    # [s, po, t2, j, f] -> [s, t2, f, j, po] -> [64, 512, 256], then dequant
    o = out_dev.transpose(0, 2, 4, 3, 1).reshape(TPC, N_POINTS, D_OUT).astype(np.float32)
    return o / so_core[:, None, None].astype(np.float32)


def _prepare(x, indices, weight, bias):
    """Shard + quantize all cores; returns (in_maps, so, colsum_ref, ref_norm)."""
    sx = _quant_x(x)
    bound = float(np.abs(weight).max()) + 1e-30
    sw = 127.0 / bound
    wg = weight[indices]                                 # [T, D_in, D_out] f32
    w8 = np.clip(np.rint(wg * sw), -127, 127).astype(np.int8)

    # per-tile out scale from input statistics
    sigx = np.sqrt((x.astype(np.float32) ** 2).mean(axis=(1, 2)))
    sigw = np.sqrt((w8.astype(np.float32) ** 2).mean(axis=(1, 2))) / sw
    sig_out = np.sqrt(D_IN) * sigx * sigw + 1e-30
    so = (127.0 / (OUT_MARGIN * sig_out)).astype(np.float32)      # [T]

    in_maps = []
    for k in range(N_CORES):
        sl = slice(k * TPC, (k + 1) * TPC)
        in_maps.append(_pack_core(x[sl], w8[sl], so[sl], sx, sw))

    # Integrity reference: column-sums are linear in the points axis, so
    # out_noBias[t].sum(axis=0) == (sum_p x_q[t]) @ w_q[t] per tile (up to
    # int8 out-quant noise). Full tile coverage at ~1% of the compute.
    xq = (x * sx).astype(E3M4).astype(np.float32) / sx
    sxq = xq.sum(axis=1)                                          # [T, D_in]
    colsum_ref = np.einsum("ti,tio->to", sxq, w8.astype(np.float32)) / sw
    ref_norm = np.linalg.norm(colsum_ref, axis=1) + 1e-6
    return in_maps, so, colsum_ref, ref_norm


def _build_clear_nc():
    """Tiny 1-DMA roundtrip program: running it has been observed to clear
    the transient NRT_EXEC_UNIT_UNRECOVERABLE device-wedge state."""
    import contextlib
    f32 = mybir.dt.float32
    nc = bass.Bass()
    a = nc.declare_dram_parameter("a", [128, 64], f32, isOutput=False)
    b = nc.declare_dram_parameter("b", [128, 64], f32, isOutput=True)
    ctx = contextlib.ExitStack()
    sb = ctx.enter_context(nc.sbuf_tensor("sb", [128, 64], f32))
    with ctx:
        with contextlib.ExitStack() as sems, nc.Block() as block:
            s1 = sems.enter_context(nc.semaphore("s1"))
            s2 = sems.enter_context(nc.semaphore("s2"))

            @block.sync
            def _(sp):
                sp.dma_start(sb[:], a[:]).then_inc(s1, 16)
                sp.wait_ge(s1, 16)
                sp.dma_start(b[:], sb[:]).then_inc(s2, 16)
    return nc


def _try_clear_device():
    try:
        if "clear_nc" not in _cache:
            _cache["clear_nc"] = _build_clear_nc()
        z = np.zeros((128, 64), dtype=np.float32)
        run_bass_kernel_spmd(_cache["clear_nc"], [{"a": z}] * N_CORES,
                             core_ids=list(range(N_CORES)))
    except Exception:  # noqa: BLE001
        pass


def kernel(x, indices, weight, bias):
    x = np.asarray(x, dtype=np.float32)
    indices = np.asarray(indices).astype(np.int64)
    weight = np.asarray(weight, dtype=np.float32)
    bias = np.asarray(bias, dtype=np.float32)

    if "nc" not in _cache:
        _cache["nc"] = _build_nc()
    nc = _cache["nc"]

    in_maps, so, colsum_ref, ref_norm = _prepare(x, indices, weight, bias)

    # retry: the remote device occasionally hits a transient failure —
    # either an NRT error (exception) or, rarely, corrupted output blocks
    last_err = None
    out = None
    for attempt in range(4):
        try:
            res = run_bass_kernel_spmd(nc, in_maps, core_ids=list(range(N_CORES)))
        except Exception as e:  # noqa: BLE001
            last_err = e
            import time
            time.sleep(5.0 * (attempt + 1))
            _try_clear_device()
            continue
        cand = np.empty((NUM_TILES, N_POINTS, D_OUT), dtype=np.float32)
        for k in range(N_CORES):
            cand[k * TPC:(k + 1) * TPC] = _unpack_core(
                res.results[k]["out"], so[k * TPC:(k + 1) * TPC])
        per_tile_rel = np.linalg.norm(cand.sum(axis=1) - colsum_ref, axis=1) / ref_norm
        if per_tile_rel.max() < 5e-2:
            out = cand
            break
        last_err = RuntimeError(
            f"integrity check failed: max per-tile colsum rel err "
            f"{per_tile_rel.max():.3e} on tiles {np.where(per_tile_rel >= 5e-2)[0][:8]}")
    if out is None:
        raise last_err
    if np.any(bias):
        out += bias[0]
    return out


# revision 29
# speedup vs baseline: 1.1109x; 1.1109x over previous
"""Distributed Trainium2 kernel for AdaptiveLinearWithChannel (MoE-routed
batched matmul):  out[t] = x[t] @ weight[indices[t]] + bias

Expert-parallel per the sharding hint: the tile dimension is sharded
64-tiles-per-core across 8 NeuronCores; the indices gather is resolved
host-side (each core receives its 64 x tiles plus its 64 pre-gathered
weight tiles), so routing is device-local and no collectives run.

Quantization (l2 rel err 1.679e-2 vs the f32 reference; gate 2e-2):
  x   -> float8_e3m4, x2 pow2 pre-scale          (8 MiB/core, Sync q1)
  w   -> int8 in HBM (w is kaiming-UNIFORM, so int8 rel err is only
         3.9e-3 vs e3m4's 1.3e-2), CAST to bf16 during the GpSimd
         SWDGE DMA — exact, |w_int|<=127          (4 MiB/core, q0)
  out -> int8, per-tile scale so[t]=127/(4*sigma_est); the DVE/ACT
         f32->int8 converts are RNE+saturating (HW-probed)
                                                  (8 MiB/core, ACT q10)
  PE runs its bf16(lhsT) x e3m4(rhs) path with exact f32 PSUM
  accumulation (products have <=13-bit mantissas).

Why this shape (from NTFF traces of each revision):
  - PE path: 256 matmuls x 512 cols / 2.4 GHz = 54.6 us. fp8 Double
    modes need e4m3/e5m2 operands (3-bit mantissa) — precision rules
    them out, so this is the compute floor.
  - SDMA fabric path: ~425 GB/s of SBUF-side bytes, shared by all 3
    DMA queues; the w cast-DMA costs WRITE-side bytes, so the total is
    25.2 MiB ~= 59.3 us — the binding path. On-chip upcasts (DVE/ACT)
    to cut it were measured SLOWER (ACT busy grew past the PE path);
    per-core weight dedup is impossible under SPMD (one program).
  - Wall ~= 8 us preamble/issue + ~60-63 us fabric-paced stream +
    ~4 us tail + ~3 us of the fixed ~7 us semaphore-reset exit ladder.
  - PE HAM warm-up: ~12 N=512 scratch matmuls fill the DMA ramp so the
    real stream starts at 2.4 GHz (HAM un-throttles after ~3.4 us of
    sustained activity; without this the first ~5 us ran at 1.2 GHz).
  - x/w SBUF buffers are fully resident (one per super-tile, SUP=4
    tiles each): SP/GpSimd issue all in-DMAs upfront, PE waits only
    per-super DMA sems, DVE (even tiles) and ACT (odd tiles) copy
    PSUM->int8 through 4 rotating PSUM bank pairs, and out-DMAs ride
    ACT q10 (supers 0-11), SP q1 (12-14, after its x stream), with the
    last super split per-tile over ACT/SP so the drain chases copies.

A full-coverage column-sum integrity check retries the rare transient
device corruption. Block exit skips gpsimd's SWDGE dge_drain (its
w-DMAs are all consumed by compute; SP/ACT drain their own queues,
which carry every out-DMA).
"""

import numpy as np
import ml_dtypes

import concourse.bass as bass
import concourse.mybir as mybir
from concourse.bass_utils import run_bass_kernel_spmd

BF16 = ml_dtypes.bfloat16
E3M4 = ml_dtypes.float8_e3m4

N_CORES = 8
NUM_TILES = 512
N_POINTS = 512          # free dim N of each matmul
D_IN = 256              # contraction, 2 chunks of 128
D_OUT = 256             # output partitions, 2 chunks of 128
CHANNELS = 1024
TPC = NUM_TILES // N_CORES   # 64 tiles per core
SUP = 4                      # tiles per super-tile (4 KiB DMA rows)
NSUP = TPC // SUP            # 16 super-tiles per core
NBUF = 8                     # o_sb buffer sets (out pipeline depth)
# x/w are FULLY RESIDENT (one SBUF buffer per super, ~16 MiB): SP/GpSimd
# issue every in-DMA upfront with no inter-engine gating, so the in
# queues run at their solo rate early and build enough lead to absorb
# the 3-queue-contention deficit that caused mid-stream PE gaps.
GROUPS_PER_SUP = SUP * 2     # psum groups (sem_pe incs) per super-tile

OUT_MARGIN = 4.0             # int8 out scale: so[t] = 127/(margin*sigma)
PE_WARM = 12                 # N=512 scratch matmuls during the DMA ramp

_cache = {}

# odd supers are plain-int8 + on-chip upcast; engine alternates DVE/ACT.
# DISABLED (empty): a 94.6us A/B showed the upcasts push ACT to ~63us
# busy (copies 38 + casts 10 + DMA issues 9) > the 54.6us PE path, so
# the whole stream stretched; the SWDGE cast-DMA fabric cost is cheaper.
# ALSO measured slower (83.3us vs 80.2): 8 front-loaded DVE upcasts make
# DVE the first-half pacer (2 copies + 1 cast = 3.79us/super > PE's 3.5).
CONV_SUPERS = []
CONV_ENGINE = {s: 'v' for s in CONV_SUPERS}
CONV_IDX = {s: i for i, s in enumerate(CONV_SUPERS)}

# out-DMA routing: super -> issuing engine; last super split per-tile
OUT_OWNER = {s: 'a' for s in range(12)}
OUT_OWNER.update({12: 's', 13: 's', 14: 's'})
TAIL_OWNER = ['a', 's', 's', 'a']            # t2 -> engine for s=15 (HWDGE only)


def _build_nc():
    bf = mybir.dt.bfloat16
    f32 = mybir.dt.float32
    e3 = mybir.dt.float8e3
    i8 = mybir.dt.int8
    nc = bass.Bass()

    # x_dev[s, p, c, t2, f]    = x[4s+t2, f, 128c+p] * SX     (e3m4)
    # w_dev[s, p, c, t2, o]    = w_int8[4s+t2, 128c+p, o]     (int8)
    # out_dev[s, po, t2, j, f] = out[4s+t2, f, 128j+po] * so[t] (int8)
    # aux_dev[p, t] = so[t]/(SX*SW)  (psum multiplier)
    x_ext = nc.declare_dram_parameter("x", [NSUP, 128, 2, SUP, N_POINTS], e3, isOutput=False)
    w_ext = nc.declare_dram_parameter("w", [NSUP, 128, 2, SUP, D_OUT], i8, isOutput=False)
    aux_ext = nc.declare_dram_parameter("aux", [128, TPC], f32, isOutput=False)
    out_ext = nc.declare_dram_parameter("out", [NSUP, 128, SUP, 2, N_POINTS], i8, isOutput=True)

    import contextlib
    ctx = contextlib.ExitStack()
    x_sb = [ctx.enter_context(nc.sbuf_tensor(f"x_sb{i}", [128, 2, SUP, N_POINTS], e3)) for i in range(NSUP)]
    w_sb = [ctx.enter_context(nc.sbuf_tensor(f"w_sb{i}", [128, 2, SUP, D_OUT], bf)) for i in range(NSUP)]
    o_sb = [ctx.enter_context(nc.sbuf_tensor(f"o_sb{i}", [128, SUP, 2, N_POINTS], i8)) for i in range(NBUF)]
    w8_sb = {c: ctx.enter_context(nc.sbuf_tensor(f"w8_sb{c}", [128, 2, SUP, D_OUT], i8)) for c in CONV_SUPERS}
    aux_sb = ctx.enter_context(nc.sbuf_tensor("aux_sb", [128, TPC], f32))
    scr_sb = ctx.enter_context(nc.sbuf_tensor("scr_sb", [128, 1], f32))
    # scratch operands for PE HAM warm-up matmuls (uninitialized is fine)
    dmw_sb = ctx.enter_context(nc.sbuf_tensor("dmw_sb", [128, 128], bf))
    dmx_sb = ctx.enter_context(nc.sbuf_tensor("dmx_sb", [128, N_POINTS], e3))
    # 4 bank-pairs: tile t uses pair t%4, one [128,1024] copy per tile
    psum = [ctx.enter_context(nc.psum_tensor(f"ps{i}", [128, 2 * N_POINTS], f32)) for i in range(4)]

    # sem_xw totals per use: plain supers get x(16)+w cast-dma(16)=32;
    # conv supers get x(16) only — the upcast completion is tracked on its
    # own engine sem (sem_cvd/sem_cva) so DMA sems never mix DGE and
    # engine increments. Slot b serves supers b, b+8 (same parity =>
    # same kind).
    def xw_total(s):
        per_use = 16 if s in CONV_IDX else 32
        return per_use * (s // NBUF + 1)

    # cast-completion threshold PE must wait for before conv super s
    def cast_wait(s):
        eng = CONV_ENGINE[s]
        cnt = sum(1 for c in CONV_SUPERS if c <= s and CONV_ENGINE[c] == eng)
        return eng, cnt

    with ctx:
        with (
            contextlib.ExitStack() as sems,
            nc.Block(no_gpsimd_drain=True) as block,
        ):
            sem_xw = [sems.enter_context(nc.semaphore(f"sem_xw{b}")) for b in range(NSUP)]
            sem_w8 = {c: sems.enter_context(nc.semaphore(f"sem_w8{c}")) for c in CONV_SUPERS}
            sem_o = [sems.enter_context(nc.semaphore(f"sem_o{b}")) for b in range(NBUF)]
            sem_aux = sems.enter_context(nc.semaphore("sem_aux"))
            sem_pe = sems.enter_context(nc.semaphore("sem_pe"))
            sem_dve = sems.enter_context(nc.semaphore("sem_dve"))
            sem_acp = sems.enter_context(nc.semaphore("sem_acp"))
            sem_cvd = sems.enter_context(nc.semaphore("sem_cvd"))   # DVE upcasts
            sem_cva = sems.enter_context(nc.semaphore("sem_cva"))   # ACT upcasts

            # per-tile copy completion: DVE even t2, ACT odd t2
            def copy_done(t):
                return (sem_dve, t // 2 + 1) if t % 2 == 0 else (sem_acp, t // 2 + 1)

            def super_copies_done(eng, s):
                eng.wait_ge(sem_dve, (SUP // 2) * (s + 1))
                eng.wait_ge(sem_acp, (SUP // 2) * (s + 1))

            # upcast completion for conv index i (0-based over CONV_SUPERS)
            def conv_done(i):
                s = CONV_SUPERS[i]
                eng = CONV_ENGINE[s]
                cnt = sum(1 for c in CONV_SUPERS[:i + 1] if CONV_ENGINE[c] == eng)
                return (sem_cvd if eng == 'v' else sem_cva, cnt)

            # upcasts are placed ~2 supers ahead of PE's need: conv super c
            # is cast while the engine sits at loop step s = max(0, c - 2)
            def conv_at(s, eng):
                for c in CONV_SUPERS:
                    if CONV_ENGINE[c] == eng and max(0, c - 2) == s:
                        yield c

            @block.sync
            def _(sp):
                for s in range(NSUP):
                    sp.dma_start(x_sb[s][:], x_ext[s]).then_inc(sem_xw[s], 16)
                for s in (12, 13, 14):
                    super_copies_done(sp, s)
                    sp.dma_start(out_ext[s], o_sb[s % NBUF][:]).then_inc(sem_o[s % NBUF], 16)
                s = NSUP - 1
                for t2 in range(SUP):
                    if TAIL_OWNER[t2] != 's':
                        continue
                    t = s * SUP + t2
                    csem, ccnt = copy_done(t)
                    sp.wait_ge(csem, ccnt)
                    sp.dma_start(out_ext[s][:, t2], o_sb[s % NBUF][:, t2]
                                 ).then_inc(sem_o[s % NBUF], 16)


            @block.gpsimd
            def _(gp):
                gp.dma_start(aux_sb[:], aux_ext[:]).then_inc(sem_aux, 16)
                for s in range(NSUP):
                    if s in CONV_IDX:
                        continue
                    gp.dma_start(w_sb[s][:], w_ext[s]).then_inc(sem_xw[s], 16)


            @block.tensor
            def _(pe):
                # HAM warm-up: PE clock ramps 1.2->2.4 GHz after ~3.4us of
                # sustained activity; fill the DMA ramp (~5.5us) with long
                # scratch matmuls so the real stream starts warm.
                for _k in range(PE_WARM):
                    pe.matmul(psum[3][:, 0:512], dmw_sb[:], dmx_sb[:],
                              start=True, stop=True, skip_group_check=True)
                for s in range(NSUP):
                    pe.wait_ge(sem_xw[s], 16 if s in CONV_IDX else 32)
                    if s in CONV_IDX:
                        pe.wait_ge(sem_cvd, CONV_IDX[s] + 1)
                    for t2 in range(SUP):
                        t = s * SUP + t2
                        if t >= 4:
                            csem, ccnt = copy_done(t - 4)
                            pe.wait_ge(csem, ccnt)
                        ps = psum[t % 4]
                        for j in range(2):
                            pe.matmul(ps[:, j * 512:(j + 1) * 512],
                                      w_sb[s][:, 0, t2, j * 128:(j + 1) * 128],
                                      x_sb[s][:, 0, t2, :], start=True, stop=False)
                            pe.matmul(ps[:, j * 512:(j + 1) * 512],
                                      w_sb[s][:, 1, t2, j * 128:(j + 1) * 128],
                                      x_sb[s][:, 1, t2, :], start=False, stop=True
                                      ).then_inc(sem_pe, 1)

            @block.vector
            def _(dve):
                dve.wait_ge(sem_aux, 16)
                for s in range(NSUP):
                    b = s % NBUF
                    for c in conv_at(s, 'v'):
                        dve.wait_ge(sem_w8[c], 16)
                        dve.tensor_copy(w_sb[c][:], w8_sb[c][:]).then_inc(sem_cvd, 1)
                    if s >= NBUF:
                        dve.wait_ge(sem_o[b], 16)
                    for t2 in range(0, SUP, 2):
                        t = s * SUP + t2
                        dve.wait_ge(sem_pe, 2 * t + 2)
                        dve.tensor_scalar_mul(o_sb[b][:, t2, :, :], psum[t % 4][:],
                                              aux_sb[:, t:t + 1]).then_inc(sem_dve, 1)

            @block.scalar
            def _(act):
                # plain-int8 staging DMAs for the DVE-upcast supers ride
                # ACT's queue, which otherwise idles until the first out
                for c in CONV_SUPERS:
                    act.dma_start(w8_sb[c][:], w_ext[c]).then_inc(sem_w8[c], 16)
                act.wait_ge(sem_aux, 16)
                # dummy activation: pulls the lazy ACT_TABLE_LOAD into the
                # preamble instead of serializing before the first copy
                act.activation(scr_sb[:], aux_sb[:, 0:1],
                               mybir.ActivationFunctionType.Copy)
                for s in range(NSUP):
                    b = s % NBUF
                    if s >= NBUF:
                        act.wait_ge(sem_o[b], 16)
                    for t2 in range(1, SUP, 2):
                        t = s * SUP + t2
                        act.wait_ge(sem_pe, 2 * t + 2)
                        act.activation(o_sb[b][:, t2, :, :], psum[t % 4][:],
                                       mybir.ActivationFunctionType.Copy,
                                       scale=aux_sb[:, t:t + 1]).then_inc(sem_acp, 1)
                    if OUT_OWNER.get(s) == 'a':
                        act.wait_ge(sem_acp, (SUP // 2) * (s + 1))
                        act.wait_ge(sem_dve, (SUP // 2) * (s + 1))
                        act.dma_start(out_ext[s], o_sb[b][:]).then_inc(sem_o[b], 16)
                s = NSUP - 1
                for t2 in range(SUP):
                    if TAIL_OWNER[t2] != 'a':
                        continue
                    t = s * SUP + t2
                    csem, ccnt = copy_done(t)
                    act.wait_ge(csem, ccnt)
                    act.dma_start(out_ext[s][:, t2], o_sb[s % NBUF][:, t2]
                                  ).then_inc(sem_o[s % NBUF], 16)

    return nc


def _quant_x(x_f32):
    """Pick a pow2 pre-scale keeping |x*SX| comfortably under e3m4 max."""
    amax = float(np.abs(x_f32).max()) + 1e-30
    return 2.0 ** int(np.floor(np.log2(14.0 / amax)))


def _pack_core(x_core_f32, w_gathered_i8, so_core, sx, sw):
    """Host-side repack of one core's shard into the device in_map."""
    x8 = (x_core_f32 * sx).astype(E3M4)                # [64, 512, 256]
    x_dev = np.ascontiguousarray(
        x8.reshape(NSUP, SUP, N_POINTS, 2, 128).transpose(0, 4, 3, 1, 2))
    w_dev = np.ascontiguousarray(
        w_gathered_i8.reshape(NSUP, SUP, 2, 128, D_OUT).transpose(0, 3, 2, 1, 4))
    aux = np.ascontiguousarray(
        np.broadcast_to((so_core / (sx * sw))[None, :], (128, TPC))).astype(np.float32)
    return {"x": x_dev, "w": w_dev, "aux": aux}


def _unpack_core(out_dev, so_core):
    # [s, po, t2, j, f] -> [s, t2, f, j, po] -> [64, 512, 256], then dequant
    o = out_dev.transpose(0, 2, 4, 3, 1).reshape(TPC, N_POINTS, D_OUT).astype(np.float32)
    return o / so_core[:, None, None].astype(np.float32)


def _prepare(x, indices, weight, bias):
    """Shard + quantize all cores; returns (in_maps, so, colsum_ref, ref_norm)."""
    sx = _quant_x(x)
    bound = float(np.abs(weight).max()) + 1e-30
    sw = 127.0 / bound
    wg = weight[indices]                                 # [T, D_in, D_out] f32
    w8 = np.clip(np.rint(wg * sw), -127, 127).astype(np.int8)

    # per-tile out scale from input statistics
    sigx = np.sqrt((x.astype(np.float32) ** 2).mean(axis=(1, 2)))
    sigw = np.sqrt((w8.astype(np.float32) ** 2).mean(axis=(1, 2))) / sw
    sig_out = np.sqrt(D_IN) * sigx * sigw + 1e-30
    so = (127.0 / (OUT_MARGIN * sig_out)).astype(np.float32)      # [T]

    in_maps = []
    for k in range(N_CORES):
        sl = slice(k * TPC, (k + 1) * TPC)
        in_maps.append(_pack_core(x[sl], w8[sl], so[sl], sx, sw))

    # Integrity reference: column-sums are linear in the points axis, so
    # out_noBias[t].sum(axis=0) == (sum_p x_q[t]) @ w_q[t] per tile (up to
    # int8 out-quant noise). Full tile coverage at ~1% of the compute.
    xq = (x * sx).astype(E3M4).astype(np.float32) / sx
    sxq = xq.sum(axis=1)                                          # [T, D_in]
    colsum_ref = np.einsum("ti,tio->to", sxq, w8.astype(np.float32)) / sw
    ref_norm = np.linalg.norm(colsum_ref, axis=1) + 1e-6
    return in_maps, so, colsum_ref, ref_norm


def _build_clear_nc():
    """Tiny 1-DMA roundtrip program: running it has been observed to clear
    the transient NRT_EXEC_UNIT_UNRECOVERABLE device-wedge state."""
    import contextlib
    f32 = mybir.dt.float32
    nc = bass.Bass()
    a = nc.declare_dram_parameter("a", [128, 64], f32, isOutput=False)
    b = nc.declare_dram_parameter("b", [128, 64], f32, isOutput=True)
    ctx = contextlib.ExitStack()
    sb = ctx.enter_context(nc.sbuf_tensor("sb", [128, 64], f32))
    with ctx:
        with contextlib.ExitStack() as sems, nc.Block() as block:
            s1 = sems.enter_context(nc.semaphore("s1"))
            s2 = sems.enter_context(nc.semaphore("s2"))

            @block.sync
            def _(sp):
                sp.dma_start(sb[:], a[:]).then_inc(s1, 16)
                sp.wait_ge(s1, 16)
                sp.dma_start(b[:], sb[:]).then_inc(s2, 16)
    return nc


def _try_clear_device():
    try:
        if "clear_nc" not in _cache:
            _cache["clear_nc"] = _build_clear_nc()
        z = np.zeros((128, 64), dtype=np.float32)
        run_bass_kernel_spmd(_cache["clear_nc"], [{"a": z}] * N_CORES,
                             core_ids=list(range(N_CORES)))
    except Exception:  # noqa: BLE001
        pass


def kernel(x, indices, weight, bias):
    x = np.asarray(x, dtype=np.float32)
    indices = np.asarray(indices).astype(np.int64)
    weight = np.asarray(weight, dtype=np.float32)
    bias = np.asarray(bias, dtype=np.float32)

    if "nc" not in _cache:
        _cache["nc"] = _build_nc()
    nc = _cache["nc"]

    in_maps, so, colsum_ref, ref_norm = _prepare(x, indices, weight, bias)

    # retry: the remote device occasionally hits a transient failure —
    # either an NRT error (exception) or, rarely, corrupted output blocks
    last_err = None
    out = None
    for attempt in range(4):
        try:
            res = run_bass_kernel_spmd(nc, in_maps, core_ids=list(range(N_CORES)))
        except Exception as e:  # noqa: BLE001
            last_err = e
            import time
            time.sleep(5.0 * (attempt + 1))
            _try_clear_device()
            continue
        cand = np.empty((NUM_TILES, N_POINTS, D_OUT), dtype=np.float32)
        for k in range(N_CORES):
            cand[k * TPC:(k + 1) * TPC] = _unpack_core(
                res.results[k]["out"], so[k * TPC:(k + 1) * TPC])
        per_tile_rel = np.linalg.norm(cand.sum(axis=1) - colsum_ref, axis=1) / ref_norm
        if per_tile_rel.max() < 5e-2:
            out = cand
            break
        last_err = RuntimeError(
            f"integrity check failed: max per-tile colsum rel err "
            f"{per_tile_rel.max():.3e} on tiles {np.where(per_tile_rel >= 5e-2)[0][:8]}")
    if out is None:
        raise last_err
    if np.any(bias):
        out += bias[0]
    return out
